# revision 2
# baseline (speedup 1.0000x reference)
"""Trainium2 Bass kernel for nn_FB_GCN (2x 2-layer GCN + attention fusion +
3 contrastive losses over dense NxN adjacency masks + dim-label loss).

Self-contained: host-side sharding/layout prep + an 8-core SPMD Bass/Tile
kernel. Data-parallel over node rows; edge aggregation via one-hot
scatter-matmuls on the tensor engine; NxN adjacency matrices streamed
row-block-wise (bf16) against on-chip exp(sim) tiles.
"""
import numpy as np
import ml_dtypes

BF16 = ml_dtypes.bfloat16

# problem constants (hardcoded per contest rules)
N = 8192
E = 131072
IN, HID, OUT = 512, 512, 256
ATT_H = 16
LAM, ALPHA = 0.5, 0.1
SIGMA = 1e-10
NC_ = 8            # cores
ROWS = N // NC_    # 1024 rows per core
NT = ROWS // 128   # 8 node tiles per core

_cache = {}


# ---------------------------------------------------------------- host prep
def _wrap_idx(idx):
    """dma_gather index layout: idx i at [i%16, i//16], replicated to 128 parts."""
    n = len(idx)
    assert n % 16 == 0
    w = np.asarray(idx, np.int16).reshape(n // 16, 16).T  # [16, n/16]
    return np.tile(w, (8, 1))  # [128, n/16]


def _shard_edges(src, dst):
    """Shard edges by dst row-block/tile. Returns per-core
    (src_ids[8 tiles -> list], dst_local[...]) before padding."""
    out = []
    for c in range(NC_):
        m = (dst // ROWS) == c
        es, ed = src[m], dst[m] - c * ROWS
        tiles = []
        for t in range(NT):
            tm = (ed // 128) == t
            tiles.append((es[tm].astype(np.int64), (ed[tm] - t * 128).astype(np.int64)))
        out.append(tiles)
    return out


def _prep_graph(edge_index):
    """Build per-core gather-index + one-hot-id arrays for one graph."""
    src = np.asarray(edge_index[0], np.int64)
    dst = np.asarray(edge_index[1], np.int64)

    dshard = _shard_edges(src, dst)            # aggregation shard (by dst)
    sshard = _shard_edges(dst, src)            # degree-out shard (by src); ids = src_local
    # NOTE: _shard_edges(a, b) shards by b and returns (a_vals, b_local).
    # For sshard we pass (dst, src) so it shards by src; we only use b_local.

    et_d = max(max(len(t[0]) for t in core) for core in dshard)
    et_s = max(max(len(t[0]) for t in core) for core in sshard)
    et_d = max(128, -(-et_d // 128) * 128)
    et_s = max(128, -(-et_s // 128) * 128)

    g = dict(et_d=et_d, et_s=et_s, nb_d=et_d // 128, nb_s=et_s // 128)
    g["src_idx"] = []   # [128, NT*et_d/16] int16 per core (gather indices)
    g["dst_ids"] = []   # [128, NT*nb_d] f32 per core (one-hot ids, pad -1)
    g["srcp_ids"] = []  # [128, NT*nb_s] f32 per core (src-local ids for deg_out)
    for c in range(NC_):
        idx_cols, id_cols, sid_cols = [], [], []
        for t in range(NT):
            es, edl = dshard[c][t]
            pad = et_d - len(es)
            es_p = np.concatenate([es, np.zeros(pad, np.int64)])
            id_p = np.concatenate([edl, -np.ones(pad, np.int64)])
            idx_cols.append(_wrap_idx(es_p))
            id_cols.append(id_p.astype(np.float32).reshape(g["nb_d"], 128).T)
            _, sl = sshard[c][t]
            pads = et_s - len(sl)
            sl_p = np.concatenate([sl, -np.ones(pads, np.int64)])
            sid_cols.append(sl_p.astype(np.float32).reshape(g["nb_s"], 128).T)
        g["src_idx"].append(np.concatenate(idx_cols, axis=1))
        g["dst_ids"].append(np.concatenate(id_cols, axis=1))
        g["srcp_ids"].append(np.concatenate(sid_cols, axis=1))
    return g


# ---------------------------------------------------------------- device kernel
def _build(nb_a_d, nb_a_s, nb_x_d, nb_x_s, debug=False, stage=99):
    import concourse.bacc as bacc
    import concourse.mybir as mybir
    import concourse.tile as tile
    from concourse.dve_ops import TENSOR_TENSOR_REDUCE

    dt = mybir.dt
    AF = mybir.ActivationFunctionType
    AL = mybir.AluOpType

    nc = bacc.Bacc(None, num_devices=NC_)

    # ---------------- I/O -----------------
    feat_in = nc.dram_tensor("feat_bf", [N, IN], dt.bfloat16, kind="ExternalInput")
    xblk_in = nc.dram_tensor("xblk", [ROWS, IN], dt.bfloat16, kind="ExternalInput")
    adj_in = {k: nc.dram_tensor(f"adj_{k}", [ROWS, N], dt.bfloat16, kind="ExternalInput")
              for k in ("label", "X", "rec")}
    gi = {}
    for gname, nb_d, nb_s in (("a", nb_a_d, nb_a_s), ("x", nb_x_d, nb_x_s)):
        gi[gname] = dict(
            nb_d=nb_d, nb_s=nb_s,
            src_idx=nc.dram_tensor(f"srcidx_{gname}", [128, NT * nb_d * 8], dt.int16,
                                   kind="ExternalInput"),
            dst_ids=nc.dram_tensor(f"dstid_{gname}", [128, NT * nb_d], dt.float32,
                                   kind="ExternalInput"),
            srcp_ids=nc.dram_tensor(f"srcpid_{gname}", [128, NT * nb_s], dt.float32,
                                    kind="ExternalInput"),
            W0=nc.dram_tensor(f"W0{gname}", [IN, HID], dt.bfloat16, kind="ExternalInput"),
            W1=nc.dram_tensor(f"W1{gname}", [HID, OUT], dt.bfloat16, kind="ExternalInput"),
            b0=nc.dram_tensor(f"b0{gname}", [1, HID], dt.bfloat16, kind="ExternalInput"),
            b1=nc.dram_tensor(f"b1{gname}", [1, OUT], dt.bfloat16, kind="ExternalInput"),
        )
    wp1_in = nc.dram_tensor("Wp1", [OUT, ATT_H], dt.bfloat16, kind="ExternalInput")
    bp1_in = nc.dram_tensor("bp1", [1, ATT_H], dt.bfloat16, kind="ExternalInput")
    wp2_in = nc.dram_tensor("wp2", [ATT_H, 1], dt.bfloat16, kind="ExternalInput")
    iota_in = nc.dram_tensor("iota", [128, 128], dt.bfloat16, kind="ExternalInput")
    idbf_in = nc.dram_tensor("idbf", [128, 128], dt.bfloat16, kind="ExternalInput")

    out_t = nc.dram_tensor("out", [128, 8], dt.float32, kind="ExternalOutput")
    if debug:
        dbg = {
            "deg": nc.dram_tensor("dbg_deg", [128, 32], dt.float32, kind="ExternalOutput"),
            "h1w": nc.dram_tensor("dbg_h1w", [2, ROWS, OUT], dt.float32, kind="ExternalOutput"),
            "h2": nc.dram_tensor("dbg_h2", [2, ROWS, OUT], dt.float32, kind="ExternalOutput"),
            "hf": nc.dram_tensor("dbg_hf", [ROWS, OUT], dt.float32, kind="ExternalOutput"),
            "beta": nc.dram_tensor("dbg_beta", [128, 8], dt.float32, kind="ExternalOutput"),
            "pt": nc.dram_tensor("dbg_pt", [3, 2, 128, 8], dt.float32, kind="ExternalOutput"),
            "dc": nc.dram_tensor("dbg_dc", [4, 128, 256], dt.float32, kind="ExternalOutput"),
            "pt2": nc.dram_tensor("dbg_pt2", [2, 128, 8], dt.float32, kind="ExternalOutput"),
        }

    # collective buffers (single-use, Shared)
    ns_ag_in = {g: nc.dram_tensor(f"nsag_in_{g}", [128, 8], dt.float32, kind="Internal")
                for g in ("a", "x")}
    ns_ag_out = {g: nc.dram_tensor(f"nsag_out_{g}", [NC_, 128, 8], dt.float32,
                                   kind="Internal", addr_space="Shared") for g in ("a", "x")}
    xs_dram = {g: nc.dram_tensor(f"xs_{g}", [N, IN], dt.bfloat16, kind="Internal")
               for g in ("a", "x")}
    h1w_loc = {g: nc.dram_tensor(f"h1wloc_{g}", [ROWS, OUT], dt.bfloat16, kind="Internal")
               for g in ("a", "x")}
    h1w_full = {g: nc.dram_tensor(f"h1wfull_{g}", [NC_, ROWS, OUT], dt.bfloat16,
                                  kind="Internal", addr_space="Shared") for g in ("a", "x")}
    znt_loc = {e: nc.dram_tensor(f"zntloc_{e}", [2 * 128, ROWS], dt.bfloat16, kind="Internal")
               for e in ("za", "zx", "zf")}
    znt_full = {e: nc.dram_tensor(f"zntfull_{e}", [NC_, 2 * 128, ROWS], dt.bfloat16,
                                  kind="Internal", addr_space="Shared") for e in ("za", "zx", "zf")}
    dim_loc = nc.dram_tensor("dimloc", [4, 128, OUT + 1], dt.float32, kind="Internal")
    dim_full = nc.dram_tensor("dimfull", [4, 128, OUT + 1], dt.float32,
                              kind="Internal", addr_space="Shared")
    dw_dram = nc.dram_tensor("dw_dram", [ROWS], dt.float32, kind="Internal")
    bar_in = nc.dram_tensor("barin", [128, 1], dt.float32, kind="Internal")
    bar_out = nc.dram_tensor("barout", [128, 1], dt.float32,
                             kind="Internal", addr_space="Shared")

    RG = [list(range(NC_))]

    with tile.TileContext(nc) as tc:
        with tc.tile_pool(name="const", bufs=1) as constp, \
             tc.tile_pool(name="emb", bufs=1) as embp, \
             tc.tile_pool(name="work", bufs=2) as work, \
             tc.tile_pool(name="stat", bufs=1) as statp:

            # ---------- constants ----------
            iota_sb = constp.tile([128, 128], dt.bfloat16)
            nc.sync.dma_start(iota_sb[:], iota_in[:])
            idbf_sb = constp.tile([128, 128], dt.bfloat16)
            nc.sync.dma_start(idbf_sb[:], idbf_in[:])
            ones_col = constp.tile([128, 1], dt.bfloat16)
            nc.vector.memset(ones_col[:], 1.0)
            ones_row = constp.tile([1, 128], dt.bfloat16)
            nc.vector.memset(ones_row[:], 1.0)

            wp1_sb = constp.tile([128, 2, ATT_H], dt.bfloat16)
            nc.sync.dma_start(wp1_sb[:], wp1_in.rearrange("(kc p) a -> p kc a", p=128))
            bp1_sb = constp.tile([1, ATT_H], dt.bfloat16)
            nc.sync.dma_start(bp1_sb[:], bp1_in[:])
            wp2_sb = constp.tile([16, 1], dt.bfloat16)
            nc.sync.dma_start(wp2_sb[:], wp2_in[:])

            xblk_sb = constp.tile([128, NT, IN], dt.bfloat16)
            nc.sync.dma_start(xblk_sb[:], xblk_in.rearrange("(t p) f -> p t f", p=128))

            # embedding stores (bf16 rows per node-tile)
            h2_sb = {g: embp.tile([128, NT * OUT], dt.bfloat16, name=f"h2_{g}")
                     for g in ("a", "x")}
            hf_sb = embp.tile([128, NT * OUT], dt.bfloat16)
            znt_own = {e: embp.tile([128, 2, ROWS], dt.bfloat16, name=f"zntown_{e}")
                       for e in ("za", "zx", "zf")}

            loss_parts = statp.tile([128, 8], dt.float32)
            nc.vector.memset(loss_parts[:], 0.0)

            if debug:
                deg_dbg = statp.tile([128, 32], dt.float32)

            # =======================================================
            # GCN for both graphs
            # =======================================================
            for ig, g in enumerate(("a", "x")):
                G = gi[g]
                nb_d, nb_s = G["nb_d"], G["nb_s"]

                with tc.tile_pool(name=f"gcn_{g}", bufs=1) as gp, \
                     tc.tile_pool(name=f"gath_{g}", bufs=2) as gathp, \
                     tc.tile_pool(name=f"psg_{g}", bufs=1, space="PSUM") as psg:
                    # ---- load id/idx arrays
                    dstid_sb = gp.tile([128, NT * nb_d], dt.float32)
                    nc.sync.dma_start(dstid_sb[:], G["dst_ids"][:])
                    srcpid_sb = gp.tile([128, NT * nb_s], dt.float32)
                    nc.sync.dma_start(srcpid_sb[:], G["srcp_ids"][:])
                    srcidx_sb = gp.tile([128, NT * nb_d * 8], dt.int16)
                    nc.sync.dma_start(srcidx_sb[:], G["src_idx"][:])
                    w0_sb = gp.tile([128, 4, HID], dt.bfloat16)
                    nc.sync.dma_start(w0_sb[:], G["W0"].rearrange("(kc p) f -> p kc f", p=128))
                    w1_sb = gp.tile([128, 4, OUT], dt.bfloat16)
                    nc.sync.dma_start(w1_sb[:], G["W1"].rearrange("(kc p) f -> p kc f", p=128))
                    b0_sb = gp.tile([1, HID], dt.bfloat16)
                    nc.sync.dma_start(b0_sb[:], G["b0"][:])
                    b1_sb = gp.tile([1, OUT], dt.bfloat16)
                    nc.sync.dma_start(b1_sb[:], G["b1"][:])

                    # b1 broadcast tile [128, OUT]
                    b1b_ps = psg.tile([128, OUT], dt.float32, tag="wout", bufs=2)
                    nc.tensor.matmul(b1b_ps[:], ones_row[:], b1_sb[:], start=True, stop=True)
                    b1_bcast = gp.tile([128, OUT], dt.bfloat16)
                    nc.vector.tensor_copy(b1_bcast[:], b1b_ps[:])

                    # ---- S store + degree pass
                    s_store = gp.tile([128, NT * nb_d, 128], dt.bfloat16)
                    degI_ps = psg.tile([128, NT], dt.float32, tag="deg", bufs=2)
                    degO_ps = psg.tile([128, NT], dt.float32, tag="deg", bufs=2)
                    for t in range(NT):
                        for b in range(nb_d):
                            col = t * nb_d + b
                            nc.vector.tensor_scalar(
                                out=s_store[:, col, :], in0=iota_sb[:],
                                scalar1=dstid_sb[:, col:col + 1], scalar2=None,
                                op0=AL.is_equal)
                            nc.tensor.matmul(degI_ps[:, t:t + 1], s_store[:, col, :],
                                             ones_col[:], start=(b == 0), stop=(b == nb_d - 1))
                        s_src = work.tile([128, 128], dt.bfloat16, name="s_src")
                        for b in range(nb_s):
                            col = t * nb_s + b
                            nc.vector.tensor_scalar(
                                out=s_src[:], in0=iota_sb[:],
                                scalar1=srcpid_sb[:, col:col + 1], scalar2=None,
                                op0=AL.is_equal)
                            nc.tensor.matmul(degO_ps[:, t:t + 1], s_src[:],
                                             ones_col[:], start=(b == 0), stop=(b == nb_s - 1))

                    # ---- degrees -> ns (deg_out^-1/2), nd (deg_in^-1/2)
                    def inv_sqrt_deg(deg_ps, name):
                        d = gp.tile([128, NT], dt.float32, name=f"d_{name}")
                        nc.vector.tensor_copy(d[:], deg_ps[:])
                        dm = gp.tile([128, NT], dt.float32, name=f"dm_{name}")
                        nc.vector.tensor_scalar(out=dm[:], in0=d[:], scalar1=1e-30,
                                                scalar2=None, op0=AL.max)
                        nc.scalar.activation(dm[:], dm[:], AF.Ln)
                        nc.scalar.activation(dm[:], dm[:], AF.Exp, scale=-0.5)
                        cap = gp.tile([128, NT], dt.float32, name=f"cap_{name}")
                        nc.vector.tensor_scalar(out=cap[:], in0=d[:], scalar1=1e30,
                                                scalar2=None, op0=AL.mult)
                        nc.vector.tensor_tensor(out=dm[:], in0=dm[:], in1=cap[:], op=AL.min)
                        return d, dm

                    degI, nd_t = inv_sqrt_deg(degI_ps, f"i{g}")
                    degO, ns_own = inv_sqrt_deg(degO_ps, f"o{g}")
                    if debug:
                        nc.vector.tensor_copy(deg_dbg[:, ig * 16:ig * 16 + 8], degI[:])
                        nc.vector.tensor_copy(deg_dbg[:, ig * 16 + 8:ig * 16 + 16], degO[:])

                    # ---- all-gather ns -> full [128, 64]
                    nc.sync.dma_start(ns_ag_in[g][:], ns_own[:])
                    nc.gpsimd.collective_compute(
                        "AllGather", AL.bypass, replica_groups=RG,
                        ins=[ns_ag_in[g][:]], outs=[ns_ag_out[g][:]])
                    ns_full = gp.tile([128, NC_, NT], dt.float32)
                    nc.sync.dma_start(ns_full[:],
                                      ns_ag_out[g].rearrange("c p t -> p c t"))

                    # ---- xs = feat * ns  (full, bf16) -> DRAM
                    for ft in range(64 if stage >= 2 else 0):
                        xs_t = work.tile([128, IN], dt.bfloat16, name="xs_t")
                        nc.sync.dma_start(
                            xs_t[:], feat_in[ft * 128:(ft + 1) * 128, :])
                        nc.vector.tensor_scalar(
                            out=xs_t[:], in0=xs_t[:], scalar1=ns_full[:, ft // NT, ft % NT: ft % NT + 1],
                            scalar2=None, op0=AL.mult)
                        nc.sync.dma_start(xs_dram[g][ft * 128:(ft + 1) * 128, :], xs_t[:])

                    # ---- Layer 1 (+ W1 projection) per dst tile
                    s3 = 99 if stage >= 4 else (stage - 30 if 30 <= stage < 40 else (99 if stage >= 3 else -1))
                    for t in range(NT if s3 >= 0 else 0):
                        g1 = gathp.tile([128, nb_d, IN], dt.bfloat16, name="g1")
                        for b0 in range(0, nb_d, 8):
                            nbc = min(8, nb_d - b0)
                            nc.gpsimd.dma_gather(
                                out_ap=g1[:, b0:b0 + nbc, :], in_ap=xs_dram[g][:],
                                idxs_ap=srcidx_sb[:, t * nb_d * 8 + b0 * 8:
                                                  t * nb_d * 8 + (b0 + nbc) * 8],
                                num_idxs=nbc * 128, num_idxs_reg=nbc * 128, elem_size=IN)
                        if s3 < 1:
                            continue
                        agg_ps = psg.tile([128, IN], dt.float32, name="agg_ps", tag="agg", bufs=2)
                        for b in range(nb_d):
                            nc.tensor.matmul(agg_ps[:], s_store[:, t * nb_d + b, :],
                                             g1[:, b, :], start=(b == 0), stop=(b == nb_d - 1))
                        aggn = work.tile([128, IN], dt.bfloat16, name="aggn")
                        nc.scalar.activation(aggn[:], agg_ps[:], AF.Copy,
                                             scale=nd_t[:, t:t + 1])
                        # transpose aggn -> 4 chunks [128fin, 128dst]
                        if s3 < 2:
                            continue
                        h1_ps = psg.tile([128, HID], dt.float32, name="h1_ps", tag="wout", bufs=2)
                        for kc in range(4):
                            tr_ps = psg.tile([128, 128], dt.bfloat16, name="tr_ps", tag="tr", bufs=2)
                            nc.tensor.transpose(tr_ps[:], aggn[:, kc * 128:(kc + 1) * 128],
                                                idbf_sb[:])
                            trsb = work.tile([128, 128], dt.bfloat16, name="trsb")
                            nc.vector.tensor_copy(trsb[:], tr_ps[:])
                            nc.tensor.matmul(h1_ps[:], trsb[:], w0_sb[:, kc, :],
                                             start=(kc == 0), stop=False)
                        nc.tensor.matmul(h1_ps[:], ones_row[:], b0_sb[:],
                                         start=False, stop=True)
                        h1s = work.tile([128, HID], dt.bfloat16, name="h1s")
                        nc.scalar.activation(h1s[:], h1_ps[:], AF.Relu,
                                             scale=ns_own[:, t:t + 1])
                        # h1w = h1s @ W1
                        if s3 < 3:
                            continue
                        h1w_ps = psg.tile([128, OUT], dt.float32, name="h1w_ps", tag="wout", bufs=2)
                        for kc in range(4):
                            tr2_ps = psg.tile([128, 128], dt.bfloat16, name="tr2_ps", tag="tr", bufs=2)
                            nc.tensor.transpose(tr2_ps[:], h1s[:, kc * 128:(kc + 1) * 128],
                                                idbf_sb[:])
                            tr2sb = work.tile([128, 128], dt.bfloat16, name="tr2sb")
                            nc.vector.tensor_copy(tr2sb[:], tr2_ps[:])
                            nc.tensor.matmul(h1w_ps[:], tr2sb[:], w1_sb[:, kc, :],
                                             start=(kc == 0), stop=(kc == 3))
                        if s3 < 4:
                            continue
                        h1w_sb = work.tile([128, OUT], dt.bfloat16, name="h1w_sb")
                        nc.scalar.activation(h1w_sb[:], h1w_ps[:], AF.Copy)
                        nc.sync.dma_start(h1w_loc[g][t * 128:(t + 1) * 128, :], h1w_sb[:])
                        if debug:
                            h1wd = work.tile([128, OUT], dt.float32, name="h1wd")
                            nc.vector.tensor_copy(h1wd[:], h1w_ps[:])
                            nc.sync.dma_start(dbg["h1w"][ig, t * 128:(t + 1) * 128, :], h1wd[:])

                    # ---- all-gather h1w
                    if stage >= 3:
                        nc.gpsimd.collective_compute(
                            "AllGather", AL.bypass, replica_groups=RG,
                            ins=[h1w_loc[g][:]], outs=[h1w_full[g][:]])

                    # ---- Layer 2 per dst tile
                    h1w_view = h1w_full[g].rearrange("c r f -> (c r) f")
                    for t in range(NT if stage >= 4 else 0):
                        g2 = gathp.tile([128, nb_d, OUT], dt.bfloat16, name="g2")
                        for b0 in range(0, nb_d, 8):
                            nbc = min(8, nb_d - b0)
                            nc.gpsimd.dma_gather(
                                out_ap=g2[:, b0:b0 + nbc, :], in_ap=h1w_view,
                                idxs_ap=srcidx_sb[:, t * nb_d * 8 + b0 * 8:
                                                  t * nb_d * 8 + (b0 + nbc) * 8],
                                num_idxs=nbc * 128, num_idxs_reg=nbc * 128, elem_size=OUT)
                        agg2_ps = psg.tile([128, OUT], dt.float32, name="agg2_ps", tag="agg", bufs=2)
                        for b in range(nb_d):
                            nc.tensor.matmul(agg2_ps[:], s_store[:, t * nb_d + b, :],
                                             g2[:, b, :], start=(b == 0), stop=(b == nb_d - 1))
                        h2t = work.tile([128, OUT], dt.bfloat16, name="h2t")
                        nc.scalar.activation(h2t[:], agg2_ps[:], AF.Copy,
                                             scale=nd_t[:, t:t + 1])
                        nc.vector.tensor_tensor(
                            out=h2_sb[g][:, t * OUT:(t + 1) * OUT], in0=h2t[:],
                            in1=b1_bcast[:], op=AL.add)

            if debug:
                nc.sync.dma_start(dbg["deg"][:], deg_dbg[:])
            if stage < 4:
                for g in ("a", "x"):
                    nc.vector.memset(h2_sb[g][:], 0.01)
            if debug:
                for ig, g in enumerate(("a", "x")):
                    for t in range(NT):
                        h2d = work.tile([128, OUT], dt.float32, name="h2d")
                        nc.vector.tensor_copy(h2d[:], h2_sb[g][:, t * OUT:(t + 1) * OUT])
                        nc.sync.dma_start(dbg["h2"][ig, t * 128:(t + 1) * 128, :], h2d[:])

            # =======================================================
            # Attention fusion
            # =======================================================
            if stage < 5:
                nc.vector.memset(hf_sb[:], 0.01)
            with tc.tile_pool(name="fuse", bufs=1) as fp, \
                 tc.tile_pool(name="psf", bufs=1, space="PSUM") as psf:
              if stage >= 5:
                  w_rows = fp.tile([1, 2 * ROWS // 128 * 128], dt.float32)  # [1, 2048]: wx | wadj
                  for ib, g in enumerate(("x", "a")):
                      for t in range(NT):
                          t1_ps = psf.tile([16, 128], dt.float32, name="t1_ps", tag="t1w", bufs=2)
                          for kc in range(2):
                              trh_ps = psf.tile([128, 128], dt.bfloat16, name="trh_ps", tag="trh", bufs=2)
                              nc.tensor.transpose(
                                  trh_ps[:], h2_sb[g][:, t * OUT + kc * 128: t * OUT + kc * 128 + 128],
                                  idbf_sb[:])
                              trh = work.tile([128, 128], dt.bfloat16, name="trh")
                              nc.vector.tensor_copy(trh[:], trh_ps[:])
                              nc.tensor.matmul(t1_ps[:], wp1_sb[:, kc, :],
                                               trh[:], start=(kc == 0), stop=False)
                          nc.tensor.matmul(t1_ps[:], bp1_sb[:], ones_row[:],
                                           start=False, stop=True)
                          t1_sb = work.tile([16, 128], dt.bfloat16, name="t1_sb")
                          nc.scalar.activation(t1_sb[:], t1_ps[:], AF.Tanh)
                          w_ps = psf.tile([1, 128], dt.float32, name="w_ps", tag="t1w", bufs=2)
                          nc.tensor.matmul(w_ps[:], wp2_sb[:], t1_sb[:], start=True, stop=True)
                          nc.vector.tensor_copy(
                              w_rows[:, ib * ROWS + t * 128: ib * ROWS + (t + 1) * 128], w_ps[:])
                  # beta_x = sigmoid(wx - wadj) on [1, 1024]
                  dw = fp.tile([1, ROWS], dt.float32)
                  nc.vector.tensor_tensor(out=dw[:], in0=w_rows[:, 0:ROWS],
                                          in1=w_rows[:, ROWS:2 * ROWS], op=AL.subtract)
                  nc.scalar.activation(dw[:], dw[:], AF.Exp, scale=-1.0)
                  nc.vector.tensor_scalar(out=dw[:], in0=dw[:], scalar1=1.0,
                                          scalar2=None, op0=AL.add)
                  nc.vector.reciprocal(dw[:], dw[:])
                  nc.sync.dma_start(dw_dram.rearrange("(o x) -> o x", o=1), dw[:])
                  beta_col = fp.tile([128, 1, NT], dt.float32)
                  nc.sync.dma_start(beta_col[:],
                                    dw_dram.rearrange("(t p o) -> p o t", p=128, o=1))
                  if debug:
                      nc.sync.dma_start(dbg["beta"][:], beta_col[:, 0, :])
                  # h_fuse = h_adj + beta*(h_x - h_adj)
                  for t in range(NT):
                      dhf = work.tile([128, OUT], dt.bfloat16, name="dhf")
                      nc.vector.tensor_tensor(out=dhf[:], in0=h2_sb["x"][:, t * OUT:(t + 1) * OUT],
                                              in1=h2_sb["a"][:, t * OUT:(t + 1) * OUT],
                                              op=AL.subtract)
                      nc.vector.scalar_tensor_tensor(
                          out=hf_sb[:, t * OUT:(t + 1) * OUT], in0=dhf[:],
                          scalar=beta_col[:, 0, t:t + 1], in1=h2_sb["a"][:, t * OUT:(t + 1) * OUT],
                          op0=AL.mult, op1=AL.add)
                  if debug:
                      for t in range(NT):
                          hfd = work.tile([128, OUT], dt.float32, name="hfd")
                          nc.vector.tensor_copy(hfd[:], hf_sb[:, t * OUT:(t + 1) * OUT])
                          nc.sync.dma_start(dbg["hf"][t * 128:(t + 1) * 128, :], hfd[:])

            # =======================================================
            # l2norm + transpose + all-gather z^T for the three embeddings
            # =======================================================
            with tc.tile_pool(name="znorm", bufs=2) as zp, \
                 tc.tile_pool(name="psz", bufs=1, space="PSUM") as psz:
                for e, src_sb in ((("za", h2_sb["a"]), ("zx", h2_sb["x"]), ("zf", hf_sb)) if stage >= 6 else ()):
                    for t in range(NT):
                        seg = src_sb[:, t * OUT:(t + 1) * OUT]
                        scr = zp.tile([128, OUT], dt.bfloat16, name="scr")
                        nrm2 = zp.tile([128, 1], dt.float32, name="nrm2")
                        nc.vector._custom_dve(TENSOR_TENSOR_REDUCE, out=scr[:],
                                              in0=seg, in1=seg, s0=0.0, s1=1.0,
                                              accum_out=nrm2[:])
                        nc.vector.tensor_scalar(out=nrm2[:], in0=nrm2[:], scalar1=1e-30,
                                                scalar2=None, op0=AL.max)
                        nc.scalar.activation(nrm2[:], nrm2[:], AF.Ln)
                        nc.scalar.activation(nrm2[:], nrm2[:], AF.Exp, scale=0.5)
                        nc.vector.tensor_scalar(out=nrm2[:], in0=nrm2[:], scalar1=1e-12,
                                                scalar2=None, op0=AL.max)
                        nc.vector.reciprocal(nrm2[:], nrm2[:])
                        zn_t = zp.tile([128, OUT], dt.bfloat16, name="zn_t")
                        nc.vector.tensor_scalar(out=zn_t[:], in0=seg, scalar1=nrm2[:],
                                                scalar2=None, op0=AL.mult)
                        for kc in range(2):
                            zt_ps = psz.tile([128, 128], dt.bfloat16, name="zt_ps", tag="zt", bufs=2)
                            nc.tensor.transpose(zt_ps[:], zn_t[:, kc * 128:(kc + 1) * 128],
                                                idbf_sb[:])
                            nc.vector.tensor_copy(
                                znt_own[e][:, kc, t * 128:(t + 1) * 128], zt_ps[:])
                    nc.sync.dma_start(
                        znt_loc[e].rearrange("(kc p) j -> p kc j", p=128), znt_own[e][:])
                    nc.gpsimd.collective_compute(
                        "AllGather", AL.bypass, replica_groups=RG,
                        ins=[znt_loc[e][:]], outs=[znt_full[e][:]])

            # load full z^T [128, 2, 8192] per embedding
            znt_sb = {}
            with tc.tile_pool(name="zfull", bufs=1) as zfp:
                for e in (("za", "zx", "zf") if stage >= 6 else ()):
                    znt_sb[e] = zfp.tile([128, 2, N], dt.bfloat16, name=f"zntsb_{e}")
                    for c in range(NC_):
                        nc.sync.dma_start(
                            znt_sb[e][:, :, c * ROWS:(c + 1) * ROWS],
                            znt_full[e][c].rearrange("(kc p) j -> p kc j", p=128))

                # =======================================================
                # Three contrastive losses (the heavy streaming part)
                # =======================================================
                with tc.tile_pool(name="loss", bufs=3) as lp, \
                     tc.tile_pool(name="psl", bufs=1, space="PSUM") as psl:
                    for il, (e, akey) in enumerate(((("za", "label"), ("zx", "X"), ("zf", "rec")) if stage >= 7 else ())):
                        tot_all = lp.tile([128, NT], dt.float32, name="tot_all", bufs=1)
                        pos_all = lp.tile([128, NT], dt.float32, name="pos_all", bufs=1)
                        for t in range(NT):
                            tot_cols = lp.tile([128, 16], dt.float32, name="tot_cols")
                            pos_cols = lp.tile([128, 16], dt.float32, name="pos_cols")
                            lhs0 = znt_own[e][:, 0, t * 128:(t + 1) * 128]
                            lhs1 = znt_own[e][:, 1, t * 128:(t + 1) * 128]
                            for jb in range(16):
                                sim_ps = psl.tile([128, 512], dt.float32, name="sim_ps", tag="sim", bufs=4)
                                nc.tensor.matmul(sim_ps[:], lhs0,
                                                 znt_sb[e][:, 0, jb * 512:(jb + 1) * 512],
                                                 start=True, stop=False)
                                nc.tensor.matmul(sim_ps[:], lhs1,
                                                 znt_sb[e][:, 1, jb * 512:(jb + 1) * 512],
                                                 start=False, stop=True)
                                refl = lp.tile([128, 512], dt.bfloat16, name="refl")
                                nc.scalar.activation(refl[:], sim_ps[:], AF.Exp,
                                                     accum_out=tot_cols[:, jb:jb + 1])
                                adj_t = lp.tile([128, 512], dt.bfloat16, name="adj_t")
                                nc.sync.dma_start(
                                    adj_t[:],
                                    adj_in[akey][t * 128:(t + 1) * 128, jb * 512:(jb + 1) * 512])
                                mscr = lp.tile([128, 512], dt.bfloat16, name="mscr")
                                nc.vector._custom_dve(
                                    TENSOR_TENSOR_REDUCE, out=mscr[:], in0=refl[:],
                                    in1=adj_t[:], s0=0.0, s1=1.0,
                                    accum_out=pos_cols[:, jb:jb + 1])
                            nc.vector.reduce_sum(tot_all[:, t:t + 1], tot_cols[:],
                                                 axis=mybir.AxisListType.X)
                            nc.vector.reduce_sum(pos_all[:, t:t + 1], pos_cols[:],
                                                 axis=mybir.AxisListType.X)
                        # loss partial: sum_n ln(neg+sig) - ln(pos+sig)
                        neg = lp.tile([128, NT], dt.float32, name="neg", bufs=1)
                        nc.vector.tensor_tensor(out=neg[:], in0=tot_all[:], in1=pos_all[:],
                                                op=AL.subtract)
                        if debug:
                            psd = work.tile([128, NT], dt.float32, name="psd")
                            nc.vector.tensor_copy(psd[:], pos_all[:])
                            nc.sync.dma_start(dbg["pt"][il, 0], psd[:])
                            ttd = work.tile([128, NT], dt.float32, name="ttd")
                            nc.vector.tensor_copy(ttd[:], tot_all[:])
                            nc.sync.dma_start(dbg["pt"][il, 1], ttd[:])
                        nc.vector.tensor_scalar(out=pos_all[:], in0=pos_all[:],
                                                scalar1=SIGMA, scalar2=None, op0=AL.add)
                        nc.vector.tensor_scalar(out=neg[:], in0=neg[:],
                                                scalar1=SIGMA, scalar2=None, op0=AL.add)
                        nc.scalar.activation(pos_all[:], pos_all[:], AF.Ln)
                        nc.scalar.activation(neg[:], neg[:], AF.Ln)
                        dl = lp.tile([128, NT], dt.float32, name="dl", bufs=1)
                        nc.vector.tensor_tensor(out=dl[:], in0=neg[:], in1=pos_all[:],
                                                op=AL.subtract)
                        nc.vector.reduce_sum(loss_parts[:, il:il + 1], dl[:],
                                             axis=mybir.AxisListType.X)

                # =======================================================
                # dim_lable_loss
                # =======================================================
                with tc.tile_pool(name="dim", bufs=2) as dp, \
                     tc.tile_pool(name="psd", bufs=1, space="PSUM") as psd:
                    # partial X^T Z and colsum(X) over own rows
                    hfb = dp.tile([128, NT, OUT], dt.bfloat16, bufs=1)
                    if stage < 8:
                        nc.vector.memset(hfb[:, 0, 0:1], 0.0)
                    for t in range(NT if stage >= 8 else 0):
                        nc.vector.tensor_copy(hfb[:, t, :], hf_sb[:, t * OUT:(t + 1) * OUT])
                    cs_ps = psd.tile([128, 4], dt.float32, name="cs_ps", tag="cs", bufs=1)
                    dim_sb = dp.tile([128, 4, OUT + 1], dt.float32, bufs=1)
                    if stage < 8:
                        nc.vector.memset(dim_sb[:], 0.0)
                    for mt in range(4 if stage >= 8 else 0):
                        xtz_ps = psd.tile([128, OUT], dt.float32, name="xtz_ps",
                                          tag="xtz", bufs=2)
                        for t in range(NT):
                            nc.tensor.matmul(xtz_ps[:],
                                             xblk_sb[:, t, mt * 128:(mt + 1) * 128],
                                             hfb[:, t, :], start=(t == 0), stop=(t == NT - 1))
                        for t in range(NT):
                            nc.tensor.matmul(cs_ps[:, mt:mt + 1],
                                             xblk_sb[:, t, mt * 128:(mt + 1) * 128],
                                             ones_col[:], start=(t == 0), stop=(t == NT - 1))
                        nc.vector.tensor_copy(dim_sb[:, mt, 0:OUT], xtz_ps[:])
                    if stage >= 8:
                        nc.vector.tensor_copy(dim_sb[:, :, OUT], cs_ps[:])
                    nc.sync.dma_start(dim_loc.rearrange("m p f -> p m f"), dim_sb[:])
                    nc.gpsimd.collective_compute(
                        "AllReduce", AL.add, replica_groups=RG,
                        ins=[dim_loc[:]], outs=[dim_full[:]])
                    dimf = dp.tile([128, 4, OUT + 1], dt.float32, bufs=1)
                    nc.sync.dma_start(dimf[:], dim_full.rearrange("m p f -> p m f"))

                    # dim_center rows: dc = XtZ/(colsum+eps); then l2norm rows -> dcn^T
                    dcnT = dp.tile([128, 2, 512], dt.bfloat16, bufs=1)
                    for mt in range(4 if stage >= 8 else 0):
                        csum = dp.tile([128, 1], dt.float32, name="csum")
                        nc.vector.tensor_scalar(out=csum[:], in0=dimf[:, mt, OUT:OUT + 1],
                                                scalar1=1e-5, scalar2=None, op0=AL.add)
                        nc.vector.reciprocal(csum[:], csum[:])
                        dc_t = dp.tile([128, OUT], dt.bfloat16, name="dc_t")
                        nc.vector.tensor_scalar(out=dc_t[:], in0=dimf[:, mt, 0:OUT],
                                                scalar1=csum[:], scalar2=None, op0=AL.mult)
                        if debug:
                            dcd = work.tile([128, OUT], dt.float32, name="dcd")
                            nc.vector.tensor_copy(dcd[:], dc_t[:])
                            nc.sync.dma_start(dbg["dc"][mt], dcd[:])
                        nrm2 = dp.tile([128, 1], dt.float32, name="nrm2d")
                        scr = dp.tile([128, OUT], dt.bfloat16, name="scrd")
                        nc.vector._custom_dve(TENSOR_TENSOR_REDUCE, out=scr[:],
                                              in0=dc_t[:], in1=dc_t[:], s0=0.0, s1=1.0,
                                              accum_out=nrm2[:])
                        nc.vector.tensor_scalar(out=nrm2[:], in0=nrm2[:], scalar1=1e-30,
                                                scalar2=None, op0=AL.max)
                        nc.scalar.activation(nrm2[:], nrm2[:], AF.Ln)
                        nc.scalar.activation(nrm2[:], nrm2[:], AF.Exp, scale=0.5)
                        nc.vector.tensor_scalar(out=nrm2[:], in0=nrm2[:], scalar1=1e-12,
                                                scalar2=None, op0=AL.max)
                        nc.vector.reciprocal(nrm2[:], nrm2[:])
                        nc.vector.tensor_scalar(out=dc_t[:], in0=dc_t[:], scalar1=nrm2[:],
                                                scalar2=None, op0=AL.mult)
                        for kc in range(2):
                            dct_ps = psd.tile([128, 128], dt.bfloat16, name="dct_ps", tag="dct", bufs=2)
                            nc.tensor.transpose(dct_ps[:], dc_t[:, kc * 128:(kc + 1) * 128],
                                                idbf_sb[:])
                            nc.vector.tensor_copy(dcnT[:, kc, mt * 128:(mt + 1) * 128],
                                                  dct_ps[:])

                    # refl2 = exp(zfuse_n @ dcn^T); pos/neg with X_hot mask
                    tot2 = dp.tile([128, NT], dt.float32, bufs=1)
                    pos2 = dp.tile([128, NT], dt.float32, bufs=1)
                    for t in range(NT if stage >= 8 else 0):
                        r2_ps = psd.tile([128, 512], dt.float32, name="r2_ps", tag="xtz", bufs=2)
                        nc.tensor.matmul(r2_ps[:], znt_own["zf"][:, 0, t * 128:(t + 1) * 128],
                                         dcnT[:, 0, :], start=True, stop=False)
                        nc.tensor.matmul(r2_ps[:], znt_own["zf"][:, 1, t * 128:(t + 1) * 128],
                                         dcnT[:, 1, :], start=False, stop=True)
                        refl2 = dp.tile([128, 512], dt.bfloat16, name="refl2")
                        nc.scalar.activation(refl2[:], r2_ps[:], AF.Exp,
                                             accum_out=tot2[:, t:t + 1])
                        xhot = dp.tile([128, 512], dt.bfloat16, name="xhot")
                        nc.vector.tensor_scalar(out=xhot[:], in0=xblk_sb[:, t, :],
                                                scalar1=0.0, scalar2=None, op0=AL.is_gt)
                        scr2 = dp.tile([128, 512], dt.bfloat16, name="scr2")
                        nc.vector._custom_dve(TENSOR_TENSOR_REDUCE, out=scr2[:],
                                              in0=refl2[:], in1=xhot[:], s0=0.0, s1=1.0,
                                              accum_out=pos2[:, t:t + 1])
                    if debug:
                        p2d = work.tile([128, NT], dt.float32, name="p2d")
                        nc.vector.tensor_copy(p2d[:], pos2[:])
                        nc.sync.dma_start(dbg["pt2"][0], p2d[:])
                        t2d = work.tile([128, NT], dt.float32, name="t2d")
                        nc.vector.tensor_copy(t2d[:], tot2[:])
                        nc.sync.dma_start(dbg["pt2"][1], t2d[:])
                    # loss_feat partial: -ln(pos/neg + 1e-5), pos=pos2+SIG, neg=tot2-pos2
                    if stage < 8:
                        nc.vector.memset(tot2[:], 1.0)
                        nc.vector.memset(pos2[:], 1.0)
                    neg2 = dp.tile([128, NT], dt.float32, bufs=1)
                    nc.vector.tensor_tensor(out=neg2[:], in0=tot2[:], in1=pos2[:],
                                            op=AL.subtract)
                    nc.vector.tensor_scalar(out=pos2[:], in0=pos2[:], scalar1=SIGMA,
                                            scalar2=None, op0=AL.add)
                    nc.vector.reciprocal(neg2[:], neg2[:])
                    r = dp.tile([128, NT], dt.float32, bufs=1)
                    nc.vector.tensor_tensor(out=r[:], in0=pos2[:], in1=neg2[:], op=AL.mult)
                    nc.vector.tensor_scalar(out=r[:], in0=r[:], scalar1=1e-5,
                                            scalar2=None, op0=AL.add)
                    nc.scalar.activation(r[:], r[:], AF.Ln)
                    rsum = dp.tile([128, 1], dt.float32, bufs=1)
                    nc.vector.reduce_sum(rsum[:], r[:], axis=mybir.AxisListType.X)
                    nc.vector.tensor_scalar(out=loss_parts[:, 3:4], in0=rsum[:],
                                            scalar1=-1.0, scalar2=None, op0=AL.mult)

            # ---------- output + end barrier ----------
            nc.sync.dma_start(out_t[:], loss_parts[:])
            barc = statp.tile([128, 1], dt.float32)
            nc.vector.reduce_sum(barc[:], loss_parts[:], axis=mybir.AxisListType.X)
            nc.sync.dma_start(bar_in[:], barc[:])
            nc.gpsimd.collective_compute(
                "AllReduce", AL.add, replica_groups=RG,
                ins=[bar_in[:]], outs=[bar_out[:]])

    nc.compile()
    return nc


# ---------------------------------------------------------------- entry point
def _prep(feat, adj_label, adj_X, adj_rec, W0a, b0a, W1a, b1a,
          W0x, b0x, W1x, b1x, Wp1, bp1, wp2, edge_index, edge_index_x,
          _debug=False, _stage=99):
    feat = np.asarray(feat, np.float32)
    ga = _prep_graph(np.asarray(edge_index))
    gx = _prep_graph(np.asarray(edge_index_x))

    key = (ga["nb_d"], ga["nb_s"], gx["nb_d"], gx["nb_s"], _debug, _stage)
    if key not in _cache:
        _cache[key] = _build(*key[:4], debug=_debug, stage=_stage)
    nc = _cache[key]

    feat_bf = feat.astype(BF16)
    iota = np.tile(np.arange(128, dtype=np.float32)[None, :], (128, 1)).astype(BF16)
    idbf = np.eye(128, dtype=np.float32).astype(BF16)

    base = dict(
        feat_bf=feat_bf, iota=iota, idbf=idbf,
        W0a=np.asarray(W0a, np.float32).astype(BF16),
        W1a=np.asarray(W1a, np.float32).astype(BF16),
        b0a=np.asarray(b0a, np.float32).reshape(1, HID).astype(BF16),
        b1a=np.asarray(b1a, np.float32).reshape(1, OUT).astype(BF16),
        W0x=np.asarray(W0x, np.float32).astype(BF16),
        W1x=np.asarray(W1x, np.float32).astype(BF16),
        b0x=np.asarray(b0x, np.float32).reshape(1, HID).astype(BF16),
        b1x=np.asarray(b1x, np.float32).reshape(1, OUT).astype(BF16),
        Wp1=np.asarray(Wp1, np.float32).astype(BF16),
        bp1=np.asarray(bp1, np.float32).reshape(1, ATT_H).astype(BF16),
        wp2=np.asarray(wp2, np.float32).astype(BF16),
    )
    # rename graph arrays: graph "a" uses edge_index, "x" uses edge_index_x
    adj_bf = {k: np.asarray(v, np.float32).astype(BF16)
              for k, v in (("label", adj_label), ("X", adj_X), ("rec", adj_rec))}

    in_maps = []
    for c in range(NC_):
        m = dict(base)
        m["xblk"] = feat_bf[c * ROWS:(c + 1) * ROWS]
        for k in ("label", "X", "rec"):
            m[f"adj_{k}"] = np.ascontiguousarray(adj_bf[k][c * ROWS:(c + 1) * ROWS])
        for gname, g in (("a", ga), ("x", gx)):
            m[f"srcidx_{gname}"] = np.ascontiguousarray(g["src_idx"][c])
            m[f"dstid_{gname}"] = np.ascontiguousarray(g["dst_ids"][c])
            m[f"srcpid_{gname}"] = np.ascontiguousarray(g["srcp_ids"][c])
        in_maps.append(m)

    return nc, in_maps


def kernel(_debug=False, _trace=False, _stage=99, _tmpdir=None, **inputs):
    from concourse.bass_utils import run_bass_kernel_spmd
    nc, in_maps = _prep(_debug=_debug, _stage=_stage, **inputs)
    res = run_bass_kernel_spmd(nc, in_maps, core_ids=list(range(NC_)), trace=_trace,
                               tmpdir=_tmpdir)
    parts = np.stack([r["out"] for r in res.results])  # [8, 128, 8]
    psum = parts.sum(axis=(0, 1))  # [8]
    la, lx, ladj, lf = psum[0] / N, psum[1] / N, psum[2] / N, psum[3] / N
    val = np.float32(LAM * (la + lx) + ALPHA * lf + ladj)
    if _debug or _trace:
        kernel._last = res
    return np.asarray(val, np.float32).reshape(())



# revision 16
# speedup vs baseline: 1.5703x; 1.5703x over previous
"""Trainium2 Bass kernel for nn_FB_GCN (2x 2-layer GCN + attention fusion +
3 contrastive losses over dense NxN adjacency masks + dim-label loss).

Self-contained: host-side sharding/layout prep + an 8-core SPMD Bass/Tile
kernel. Data-parallel over node rows; edge aggregation via one-hot
scatter-matmuls on the tensor engine with degree norms folded in on the
host; gathers use SWDGE prepare/trigger so descriptor generation never
blocks on the transfer; NxN adjacency matrices streamed row-block-wise
(bf16) against on-chip exp(sim) tiles.
"""
import numpy as np
import ml_dtypes

BF16 = ml_dtypes.bfloat16

# problem constants (hardcoded per contest rules)
N = 8192
E = 131072
IN, HID, OUT = 512, 512, 256
ATT_H = 16
LAM, ALPHA = 0.5, 0.1
SIGMA = 1e-10
NC_ = 8            # cores
ROWS = N // NC_    # 1024 rows per core
NT = ROWS // 128   # 8 node tiles per core
USE_PREP = False    # SWDGE prepare/trigger gathers (False: blocking dma_gather)

_cache = {}


# ---------------------------------------------------------------- host prep
def _wrap_idx(idx):
    """dma_gather index layout: idx i at [i%16, i//16], replicated to 128 parts."""
    n = len(idx)
    assert n % 16 == 0
    w = np.asarray(idx, np.int16).reshape(n // 16, 16).T  # [16, n/16]
    return np.tile(w, (8, 1))  # [128, n/16]


def _prep_graph(edge_index):
    """Shard edges by dst row-block/tile; host-precompute degree norms.

    The GraphConv norm D_dst^-1/2 A D_src^-1/2 is split as: ns[src_e] folded
    into the one-hot scatter matrix S (via sval), nd applied per dst tile.
    """
    src = np.asarray(edge_index[0], np.int64)
    dst = np.asarray(edge_index[1], np.int64)
    deg_out = np.bincount(src, minlength=N).astype(np.float64)
    deg_in = np.bincount(dst, minlength=N).astype(np.float64)
    ns = np.where(deg_out > 0, deg_out ** -0.5, 0.0).astype(np.float32)
    nd = np.where(deg_in > 0, deg_in ** -0.5, 0.0).astype(np.float32)

    percore = []
    for c in range(NC_):
        m = (dst // ROWS) == c
        es, ed = src[m], dst[m] - c * ROWS
        tiles = []
        for t in range(NT):
            tm = (ed // 128) == t
            tiles.append((es[tm], ed[tm] - t * 128))
        percore.append(tiles)

    et = max(max(len(te[0]) for te in core) for core in percore)
    et = max(128, -(-et // 128) * 128)
    nb = et // 128

    g = dict(nb=nb)
    g["src_idx"] = []   # [128, NT*nb*8] int16 per core (gather indices)
    g["dst_ids"] = []   # [128, NT*nb] f32 per core (one-hot ids, pad -1)
    g["sval"] = []      # [128, NT*nb] f32 per core (ns[src_e], pad 0)
    g["nd"] = []        # [128, NT] f32 per core (deg_in^-1/2 of own rows)
    for c in range(NC_):
        idx_cols, id_cols, sv_cols = [], [], []
        for t in range(NT):
            es, edl = percore[c][t]
            pad = et - len(es)
            es_p = np.concatenate([es, np.zeros(pad, np.int64)])
            id_p = np.concatenate([edl, -np.ones(pad, np.int64)])
            sv_p = np.concatenate([ns[es], np.zeros(pad, np.float32)])
            idx_cols.append(_wrap_idx(es_p))
            id_cols.append(id_p.astype(np.float32).reshape(nb, 128).T)
            sv_cols.append(sv_p.astype(np.float32).reshape(nb, 128).T)
        g["src_idx"].append(np.ascontiguousarray(np.concatenate(idx_cols, axis=1)))
        g["dst_ids"].append(np.ascontiguousarray(np.concatenate(id_cols, axis=1)))
        g["sval"].append(np.ascontiguousarray(np.concatenate(sv_cols, axis=1)))
        g["nd"].append(np.ascontiguousarray(
            nd[c * ROWS:(c + 1) * ROWS].reshape(NT, 128).T))
    return g


# ---------------------------------------------------------------- device kernel
def _build(nb_a, nb_x, debug=False):
    import concourse.bacc as bacc
    import concourse.mybir as mybir
    import concourse.tile as tile
    from concourse.dve_ops import TENSOR_TENSOR_REDUCE

    dt = mybir.dt
    AF = mybir.ActivationFunctionType
    AL = mybir.AluOpType

    nc = bacc.Bacc(None, num_devices=NC_)

    # ---------------- I/O -----------------
    feat_in = nc.dram_tensor("feat_bf", [N, IN], dt.bfloat16, kind="ExternalInput")
    xblk_in = nc.dram_tensor("xblk", [ROWS, IN], dt.bfloat16, kind="ExternalInput")
    adj_in = {k: nc.dram_tensor(f"adj_{k}", [ROWS, N], dt.bfloat16, kind="ExternalInput")
              for k in ("label", "X", "rec")}
    gi = {}
    for gname, nb in (("a", nb_a), ("x", nb_x)):
        gi[gname] = dict(
            nb=nb,
            src_idx=nc.dram_tensor(f"srcidx_{gname}", [128, NT * nb * 8], dt.int16,
                                   kind="ExternalInput"),
            dst_ids=nc.dram_tensor(f"dstid_{gname}", [128, NT * nb], dt.float32,
                                   kind="ExternalInput"),
            sval=nc.dram_tensor(f"sval_{gname}", [128, NT * nb], dt.float32,
                                kind="ExternalInput"),
            ndv=nc.dram_tensor(f"nd_{gname}", [128, NT], dt.float32,
                               kind="ExternalInput"),
            W0=nc.dram_tensor(f"W0{gname}", [IN, HID], dt.bfloat16, kind="ExternalInput"),
            W1=nc.dram_tensor(f"W1{gname}", [HID, OUT], dt.bfloat16, kind="ExternalInput"),
            b0=nc.dram_tensor(f"b0{gname}", [1, HID], dt.bfloat16, kind="ExternalInput"),
            b1=nc.dram_tensor(f"b1{gname}", [1, OUT], dt.bfloat16, kind="ExternalInput"),
        )
    wp1_in = nc.dram_tensor("Wp1", [OUT, ATT_H], dt.bfloat16, kind="ExternalInput")
    bp1_in = nc.dram_tensor("bp1", [1, ATT_H], dt.bfloat16, kind="ExternalInput")
    wp2_in = nc.dram_tensor("wp2", [ATT_H, 1], dt.bfloat16, kind="ExternalInput")
    iota_in = nc.dram_tensor("iota", [128, 128], dt.bfloat16, kind="ExternalInput")
    idbf_in = nc.dram_tensor("idbf", [128, 128], dt.bfloat16, kind="ExternalInput")

    out_t = nc.dram_tensor("out", [128, 8], dt.float32, kind="ExternalOutput")
    if debug:
        dbg = {
            "h1w": nc.dram_tensor("dbg_h1w", [2, ROWS, OUT], dt.float32, kind="ExternalOutput"),
            "h2": nc.dram_tensor("dbg_h2", [2, ROWS, OUT], dt.float32, kind="ExternalOutput"),
            "hf": nc.dram_tensor("dbg_hf", [ROWS, OUT], dt.float32, kind="ExternalOutput"),
            "beta": nc.dram_tensor("dbg_beta", [128, 8], dt.float32, kind="ExternalOutput"),
            "pt": nc.dram_tensor("dbg_pt", [3, 2, 128, 8], dt.float32, kind="ExternalOutput"),
            "dc": nc.dram_tensor("dbg_dc", [4, 128, 256], dt.float32, kind="ExternalOutput"),
            "pt2": nc.dram_tensor("dbg_pt2", [2, 128, 8], dt.float32, kind="ExternalOutput"),
        }

    # collective buffers (single-use, Shared)
    h1w_loc = {g: nc.dram_tensor(f"h1wloc_{g}", [ROWS, OUT], dt.bfloat16, kind="Internal")
               for g in ("a", "x")}
    h1w_full = {g: nc.dram_tensor(f"h1wfull_{g}", [NC_, ROWS, OUT], dt.bfloat16,
                                  kind="Internal", addr_space="Shared") for g in ("a", "x")}
    znt_loc = {e: nc.dram_tensor(f"zntloc_{e}", [2 * 128, ROWS], dt.bfloat16, kind="Internal")
               for e in ("za", "zx", "zf")}
    znt_full = {e: nc.dram_tensor(f"zntfull_{e}", [NC_, 2 * 128, ROWS], dt.bfloat16,
                                  kind="Internal", addr_space="Shared") for e in ("za", "zx", "zf")}
    dim_loc = nc.dram_tensor("dimloc", [4, 128, OUT + 1], dt.float32, kind="Internal")
    dim_full = nc.dram_tensor("dimfull", [4, 128, OUT + 1], dt.float32,
                              kind="Internal", addr_space="Shared")
    dw_dram = nc.dram_tensor("dw_dram", [ROWS], dt.float32, kind="Internal")
    bar_in = nc.dram_tensor("barin", [128, 1], dt.float32, kind="Internal")
    bar_out = nc.dram_tensor("barout", [128, 1], dt.float32,
                             kind="Internal", addr_space="Shared")

    RG = [list(range(NC_))]
    # One DMA-completion semaphore per DMASW lane: Tile round-robins Pool DMA
    # preps across NUM_SWDGE_GLOBAL_SEMS(=8) lanes and counts ticks per lane,
    # so each lane needs its own sem for the counts to line up.
    gsems = [nc.alloc_semaphore(f"gdma{i}") for i in range(8)]
    prep_no = [0]

    def next_gsem():
        s = gsems[prep_no[0] % 8]
        prep_no[0] += 1
        return s

    with tile.TileContext(nc) as tc:
        with tc.tile_pool(name="const", bufs=1) as constp, \
             tc.tile_pool(name="emb", bufs=1) as embp, \
             tc.tile_pool(name="work", bufs=2) as work, \
             tc.tile_pool(name="stat", bufs=1) as statp:

            # ---------- constants ----------
            iota_sb = constp.tile([128, 128], dt.bfloat16)
            nc.sync.dma_start(iota_sb[:], iota_in[:])
            idbf_sb = constp.tile([128, 128], dt.bfloat16)
            nc.sync.dma_start(idbf_sb[:], idbf_in[:])
            ones_col = constp.tile([128, 1], dt.bfloat16)
            nc.vector.memset(ones_col[:], 1.0)
            ones_row = constp.tile([1, 128], dt.bfloat16)
            nc.vector.memset(ones_row[:], 1.0)

            wp1_sb = constp.tile([128, 2, ATT_H], dt.bfloat16)
            nc.sync.dma_start(wp1_sb[:], wp1_in.rearrange("(kc p) a -> p kc a", p=128))
            bp1_sb = constp.tile([1, ATT_H], dt.bfloat16)
            nc.sync.dma_start(bp1_sb[:], bp1_in[:])
            wp2_sb = constp.tile([16, 1], dt.bfloat16)
            nc.sync.dma_start(wp2_sb[:], wp2_in[:])

            xblk_sb = constp.tile([128, NT, IN], dt.bfloat16)
            nc.sync.dma_start(xblk_sb[:], xblk_in.rearrange("(t p) f -> p t f", p=128))

            # embedding stores (bf16 rows per node-tile)
            h2_sb = {g: embp.tile([128, NT * OUT], dt.bfloat16, name=f"h2_{g}")
                     for g in ("a", "x")}
            hf_sb = embp.tile([128, NT * OUT], dt.bfloat16)
            znt_own = {e: embp.tile([128, 2, ROWS], dt.bfloat16, name=f"zntown_{e}")
                       for e in ("za", "zx", "zf")}

            loss_parts = statp.tile([128, 8], dt.float32)
            nc.vector.memset(loss_parts[:], 0.0)

            # ---------- l2norm + transpose + AG helper ----------
            def emit_znorm(e, src_sb):
                with tc.tile_pool(name=f"zn_{e}", bufs=2) as zp, \
                     tc.tile_pool(name=f"pszn_{e}", bufs=1, space="PSUM") as psz:
                    for t in range(NT):
                        seg = src_sb[:, t * OUT:(t + 1) * OUT]
                        scr = zp.tile([128, OUT], dt.bfloat16, name="scr")
                        nrm2 = zp.tile([128, 1], dt.float32, name="nrm2")
                        nc.vector._custom_dve(TENSOR_TENSOR_REDUCE, out=scr[:],
                                              in0=seg, in1=seg, s0=0.0, s1=1.0,
                                              accum_out=nrm2[:])
                        nc.vector.tensor_scalar(out=nrm2[:], in0=nrm2[:], scalar1=1e-30,
                                                scalar2=None, op0=AL.max)
                        nc.scalar.activation(nrm2[:], nrm2[:], AF.Ln)
                        nc.scalar.activation(nrm2[:], nrm2[:], AF.Exp, scale=0.5)
                        nc.vector.tensor_scalar(out=nrm2[:], in0=nrm2[:], scalar1=1e-12,
                                                scalar2=None, op0=AL.max)
                        nc.vector.reciprocal(nrm2[:], nrm2[:])
                        zn_t = zp.tile([128, OUT], dt.bfloat16, name="zn_t")
                        nc.vector.tensor_scalar(out=zn_t[:], in0=seg, scalar1=nrm2[:],
                                                scalar2=None, op0=AL.mult)
                        for kc in range(2):
                            zt_ps = psz.tile([128, 128], dt.bfloat16, name="zt_ps",
                                             tag="zt", bufs=2)
                            nc.tensor.transpose(zt_ps[:], zn_t[:, kc * 128:(kc + 1) * 128],
                                                idbf_sb[:])
                            nc.vector.tensor_copy(
                                znt_own[e][:, kc, t * 128:(t + 1) * 128], zt_ps[:])
                    nc.sync.dma_start(
                        znt_loc[e].rearrange("(kc p) j -> p kc j", p=128), znt_own[e][:])
                    nc.gpsimd.collective_compute(
                        "AllGather", AL.bypass, replica_groups=RG,
                        ins=[znt_loc[e][:]], outs=[znt_full[e][:]])

            # =======================================================
            # GCN for both graphs
            # =======================================================
            for ig, g in enumerate(("a", "x")):
                G = gi[g]
                nb = G["nb"]

                with tc.tile_pool(name=f"gcn_{g}", bufs=1) as gp, \
                     tc.tile_pool(name=f"g1_{g}", bufs=3) as g1p, \
                     tc.tile_pool(name=f"g2_{g}", bufs=3) as g2p, \
                     tc.tile_pool(name=f"psg_{g}", bufs=1, space="PSUM") as psg:
                    # ---- load id/idx arrays + weights
                    dstid_sb = gp.tile([128, NT * nb], dt.float32)
                    nc.sync.dma_start(dstid_sb[:], G["dst_ids"][:])
                    sval_sb = gp.tile([128, NT * nb], dt.float32)
                    nc.sync.dma_start(sval_sb[:], G["sval"][:])
                    nd_sb = gp.tile([128, NT], dt.float32)
                    nc.sync.dma_start(nd_sb[:], G["ndv"][:])
                    srcidx_sb = gp.tile([128, NT * nb * 8], dt.int16)
                    nc.sync.dma_start(srcidx_sb[:], G["src_idx"][:])
                    w0_sb = gp.tile([128, 4, HID], dt.bfloat16)
                    nc.sync.dma_start(w0_sb[:], G["W0"].rearrange("(kc p) f -> p kc f", p=128))
                    w1_sb = gp.tile([128, 4, OUT], dt.bfloat16)
                    nc.sync.dma_start(w1_sb[:], G["W1"].rearrange("(kc p) f -> p kc f", p=128))
                    b0_sb = gp.tile([1, HID], dt.bfloat16)
                    nc.sync.dma_start(b0_sb[:], G["b0"][:])
                    b1_sb = gp.tile([1, OUT], dt.bfloat16)
                    nc.sync.dma_start(b1_sb[:], G["b1"][:])

                    # b1 broadcast tile [128, OUT]
                    b1b_ps = psg.tile([128, OUT], dt.float32, tag="wout", bufs=2)
                    nc.tensor.matmul(b1b_ps[:], ones_row[:], b1_sb[:], start=True, stop=True)
                    b1_bcast = gp.tile([128, OUT], dt.bfloat16)
                    nc.vector.tensor_copy(b1_bcast[:], b1b_ps[:])

                    # ---- S store: (iota == dst_id) * ns[src_e], built once,
                    # used by both layers (partition = edge, free = dst node)
                    s_store = gp.tile([128, NT * nb, 128], dt.bfloat16)
                    for col in range(NT * nb):
                        nc.vector.tensor_scalar(
                            out=s_store[:, col, :], in0=iota_sb[:],
                            scalar1=dstid_sb[:, col:col + 1],
                            scalar2=sval_sb[:, col:col + 1],
                            op0=AL.is_equal, op1=AL.mult)

                    # ---- Layer 1 (+ W1 projection) per dst tile
                    for t in range(NT):
                        g1 = g1p.tile([128, nb, IN], dt.bfloat16, name="g1")
                        for b0 in range(0, nb, 8):
                            nbc = min(8, nb - b0)
                            kw = dict(
                                out_ap=g1[:, b0:b0 + nbc, :], in_ap=feat_in[:],
                                idxs_ap=srcidx_sb[:, t * nb * 8 + b0 * 8:
                                                  t * nb * 8 + (b0 + nbc) * 8],
                                num_idxs=nbc * 128, num_idxs_reg=nbc * 128,
                                elem_size=IN)
                            if USE_PREP:
                                nc.gpsimd.dma_gather(**kw, prepare_only=True, sem=next_gsem())
                                nc.gpsimd.trigger_dma(count=None)
                            else:
                                nc.gpsimd.dma_gather(**kw)
                        agg_ps = psg.tile([128, IN], dt.float32, name="agg_ps",
                                          tag="agg", bufs=2)
                        for b in range(nb):
                            nc.tensor.matmul(agg_ps[:], s_store[:, t * nb + b, :],
                                             g1[:, b, :], start=(b == 0), stop=(b == nb - 1))
                        aggn = work.tile([128, IN], dt.bfloat16, name="aggn")
                        nc.scalar.activation(aggn[:], agg_ps[:], AF.Copy,
                                             scale=nd_sb[:, t:t + 1])
                        h1_ps = psg.tile([128, HID], dt.float32, name="h1_ps",
                                         tag="wout", bufs=2)
                        for kc in range(4):
                            tr_ps = psg.tile([128, 128], dt.bfloat16, name="tr_ps",
                                             tag="tr", bufs=2)
                            nc.tensor.transpose(tr_ps[:], aggn[:, kc * 128:(kc + 1) * 128],
                                                idbf_sb[:])
                            trsb = work.tile([128, 128], dt.bfloat16, name="trsb")
                            nc.vector.tensor_copy(trsb[:], tr_ps[:])
                            nc.tensor.matmul(h1_ps[:], trsb[:], w0_sb[:, kc, :],
                                             start=(kc == 0), stop=False)
                        nc.tensor.matmul(h1_ps[:], ones_row[:], b0_sb[:],
                                         start=False, stop=True)
                        h1s = work.tile([128, HID], dt.bfloat16, name="h1s")
                        nc.scalar.activation(h1s[:], h1_ps[:], AF.Relu)
                        # h1w = relu(h1) @ W1  (ns norm lives in S now)
                        h1w_ps = psg.tile([128, OUT], dt.float32, name="h1w_ps",
                                          tag="wout", bufs=2)
                        for kc in range(4):
                            tr2_ps = psg.tile([128, 128], dt.bfloat16, name="tr2_ps",
                                              tag="tr", bufs=2)
                            nc.tensor.transpose(tr2_ps[:], h1s[:, kc * 128:(kc + 1) * 128],
                                                idbf_sb[:])
                            tr2sb = work.tile([128, 128], dt.bfloat16, name="tr2sb")
                            nc.vector.tensor_copy(tr2sb[:], tr2_ps[:])
                            nc.tensor.matmul(h1w_ps[:], tr2sb[:], w1_sb[:, kc, :],
                                             start=(kc == 0), stop=(kc == 3))
                        h1w_sb = work.tile([128, OUT], dt.bfloat16, name="h1w_sb")
                        nc.scalar.activation(h1w_sb[:], h1w_ps[:], AF.Copy)
                        nc.sync.dma_start(h1w_loc[g][t * 128:(t + 1) * 128, :], h1w_sb[:])
                        if debug:
                            h1wd = work.tile([128, OUT], dt.float32, name="h1wd")
                            nc.vector.tensor_copy(h1wd[:], h1w_ps[:])
                            nc.sync.dma_start(dbg["h1w"][ig, t * 128:(t + 1) * 128, :], h1wd[:])

                    # ---- all-gather h1w
                    nc.gpsimd.collective_compute(
                        "AllGather", AL.bypass, replica_groups=RG,
                        ins=[h1w_loc[g][:]], outs=[h1w_full[g][:]])

                    # ---- Layer 2 per dst tile
                    h1w_view = h1w_full[g].rearrange("c r f -> (c r) f")
                    for t in range(NT):
                        g2 = g2p.tile([128, nb, OUT], dt.bfloat16, name="g2")
                        for b0 in range(0, nb, 8):
                            nbc = min(8, nb - b0)
                            kw = dict(
                                out_ap=g2[:, b0:b0 + nbc, :], in_ap=h1w_view,
                                idxs_ap=srcidx_sb[:, t * nb * 8 + b0 * 8:
                                                  t * nb * 8 + (b0 + nbc) * 8],
                                num_idxs=nbc * 128, num_idxs_reg=nbc * 128,
                                elem_size=OUT)
                            if USE_PREP:
                                nc.gpsimd.dma_gather(**kw, prepare_only=True, sem=next_gsem())
                                nc.gpsimd.trigger_dma(count=None)
                            else:
                                nc.gpsimd.dma_gather(**kw)
                        agg2_ps = psg.tile([128, OUT], dt.float32, name="agg2_ps",
                                           tag="agg", bufs=2)
                        for b in range(nb):
                            nc.tensor.matmul(agg2_ps[:], s_store[:, t * nb + b, :],
                                             g2[:, b, :], start=(b == 0), stop=(b == nb - 1))
                        h2t = work.tile([128, OUT], dt.bfloat16, name="h2t")
                        nc.scalar.activation(h2t[:], agg2_ps[:], AF.Copy,
                                             scale=nd_sb[:, t:t + 1])
                        nc.vector.tensor_tensor(
                            out=h2_sb[g][:, t * OUT:(t + 1) * OUT], in0=h2t[:],
                            in1=b1_bcast[:], op=AL.add)

                # znorm + AG for this graph's embedding right away (overlaps
                # with the other graph's GCN / fusion)
                emit_znorm("za" if g == "a" else "zx", h2_sb[g])

            if debug:
                for ig, g in enumerate(("a", "x")):
                    for t in range(NT):
                        h2d = work.tile([128, OUT], dt.float32, name="h2d")
                        nc.vector.tensor_copy(h2d[:], h2_sb[g][:, t * OUT:(t + 1) * OUT])
                        nc.sync.dma_start(dbg["h2"][ig, t * 128:(t + 1) * 128, :], h2d[:])

            # =======================================================
            # Attention fusion (tanh via exp to stay on one ACT table set)
            # =======================================================
            with tc.tile_pool(name="fuse", bufs=1) as fp, \
                 tc.tile_pool(name="psf", bufs=1, space="PSUM") as psf:
                w_rows = fp.tile([1, 2 * ROWS], dt.float32)  # [1, 2048]: wx | wadj
                for ib, g in enumerate(("x", "a")):
                    for t in range(NT):
                        t1_ps = psf.tile([16, 128], dt.float32, name="t1_ps",
                                         tag="t1w", bufs=2)
                        for kc in range(2):
                            trh_ps = psf.tile([128, 128], dt.bfloat16, name="trh_ps",
                                              tag="trh", bufs=2)
                            nc.tensor.transpose(
                                trh_ps[:], h2_sb[g][:, t * OUT + kc * 128: t * OUT + kc * 128 + 128],
                                idbf_sb[:])
                            trh = work.tile([128, 128], dt.bfloat16, name="trh")
                            nc.vector.tensor_copy(trh[:], trh_ps[:])
                            nc.tensor.matmul(t1_ps[:], wp1_sb[:, kc, :],
                                             trh[:], start=(kc == 0), stop=False)
                        nc.tensor.matmul(t1_ps[:], bp1_sb[:], ones_row[:],
                                         start=False, stop=True)
                        # tanh(v) = 1 - 2/(exp(2v)+1)
                        e2 = work.tile([16, 128], dt.float32, name="e2")
                        nc.scalar.activation(e2[:], t1_ps[:], AF.Exp, scale=2.0)
                        nc.vector.tensor_scalar(out=e2[:], in0=e2[:], scalar1=1.0,
                                                scalar2=None, op0=AL.add)
                        nc.vector.reciprocal(e2[:], e2[:])
                        t1_sb = work.tile([16, 128], dt.bfloat16, name="t1_sb")
                        nc.vector.tensor_scalar(out=t1_sb[:], in0=e2[:], scalar1=-2.0,
                                                scalar2=1.0, op0=AL.mult, op1=AL.add)
                        w_ps = psf.tile([1, 128], dt.float32, name="w_ps",
                                        tag="t1w", bufs=2)
                        nc.tensor.matmul(w_ps[:], wp2_sb[:], t1_sb[:], start=True, stop=True)
                        nc.vector.tensor_copy(
                            w_rows[:, ib * ROWS + t * 128: ib * ROWS + (t + 1) * 128], w_ps[:])
                # beta_x = sigmoid(wx - wadj) on [1, 1024]
                dw = fp.tile([1, ROWS], dt.float32)
                nc.vector.tensor_tensor(out=dw[:], in0=w_rows[:, 0:ROWS],
                                        in1=w_rows[:, ROWS:2 * ROWS], op=AL.subtract)
                nc.scalar.activation(dw[:], dw[:], AF.Exp, scale=-1.0)
                nc.vector.tensor_scalar(out=dw[:], in0=dw[:], scalar1=1.0,
                                        scalar2=None, op0=AL.add)
                nc.vector.reciprocal(dw[:], dw[:])
                nc.sync.dma_start(dw_dram.rearrange("(o x) -> o x", o=1), dw[:])
                beta_col = fp.tile([128, 1, NT], dt.float32)
                nc.sync.dma_start(beta_col[:],
                                  dw_dram.rearrange("(t p o) -> p o t", p=128, o=1))
                if debug:
                    nc.sync.dma_start(dbg["beta"][:], beta_col[:, 0, :])
                # h_fuse = h_adj + beta*(h_x - h_adj)
                for t in range(NT):
                    dhf = work.tile([128, OUT], dt.bfloat16, name="dhf")
                    nc.vector.tensor_tensor(out=dhf[:], in0=h2_sb["x"][:, t * OUT:(t + 1) * OUT],
                                            in1=h2_sb["a"][:, t * OUT:(t + 1) * OUT],
                                            op=AL.subtract)
                    nc.vector.scalar_tensor_tensor(
                        out=hf_sb[:, t * OUT:(t + 1) * OUT], in0=dhf[:],
                        scalar=beta_col[:, 0, t:t + 1], in1=h2_sb["a"][:, t * OUT:(t + 1) * OUT],
                        op0=AL.mult, op1=AL.add)
                if debug:
                    for t in range(NT):
                        hfd = work.tile([128, OUT], dt.float32, name="hfd")
                        nc.vector.tensor_copy(hfd[:], hf_sb[:, t * OUT:(t + 1) * OUT])
                        nc.sync.dma_start(dbg["hf"][t * 128:(t + 1) * 128, :], hfd[:])

            emit_znorm("zf", hf_sb)

            # =======================================================
            # dim_lable_loss part 1: partial X^T Z + colsum(X), AllReduce
            # (emitted before the loss streams so the collective is hidden)
            # =======================================================
            with tc.tile_pool(name="dim", bufs=2) as dp:
              with tc.tile_pool(name="psd1", bufs=1, space="PSUM") as psd:
                hfb = dp.tile([128, NT, OUT], dt.bfloat16, bufs=1)
                for t in range(NT):
                    nc.vector.tensor_copy(hfb[:, t, :], hf_sb[:, t * OUT:(t + 1) * OUT])
                cs_ps = psd.tile([128, 4], dt.float32, name="cs_ps", tag="cs", bufs=1)
                dim_sb = dp.tile([128, 4, OUT + 1], dt.float32, bufs=1)
                for mt in range(4):
                    xtz_ps = psd.tile([128, OUT], dt.float32, name="xtz_ps",
                                      tag="xtz", bufs=2)
                    for t in range(NT):
                        nc.tensor.matmul(xtz_ps[:],
                                         xblk_sb[:, t, mt * 128:(mt + 1) * 128],
                                         hfb[:, t, :], start=(t == 0), stop=(t == NT - 1))
                    for t in range(NT):
                        nc.tensor.matmul(cs_ps[:, mt:mt + 1],
                                         xblk_sb[:, t, mt * 128:(mt + 1) * 128],
                                         ones_col[:], start=(t == 0), stop=(t == NT - 1))
                    nc.vector.tensor_copy(dim_sb[:, mt, 0:OUT], xtz_ps[:])
                nc.vector.tensor_copy(dim_sb[:, :, OUT], cs_ps[:])
                nc.sync.dma_start(dim_loc.rearrange("m p f -> p m f"), dim_sb[:])
                nc.gpsimd.collective_compute(
                    "AllReduce", AL.add, replica_groups=RG,
                    ins=[dim_loc[:]], outs=[dim_full[:]])

              # =======================================================
              # Three contrastive losses (the heavy streaming part)
              # =======================================================
              znt_sb = {}
              with tc.tile_pool(name="zfull", bufs=1) as zfp:
                for e in ("za", "zx", "zf"):
                    znt_sb[e] = zfp.tile([128, 2, N], dt.bfloat16, name=f"zntsb_{e}")
                    for c in range(NC_):
                        nc.sync.dma_start(
                            znt_sb[e][:, :, c * ROWS:(c + 1) * ROWS],
                            znt_full[e][c].rearrange("(kc p) j -> p kc j", p=128))

                with tc.tile_pool(name="loss", bufs=4) as lp, \
                     tc.tile_pool(name="psl", bufs=1, space="PSUM") as psl:
                    JW = 1024   # stream tile width (2 PSUM banks)
                    NJ = N // JW
                    for il, (e, akey) in enumerate((("za", "label"), ("zx", "X"),
                                                    ("zf", "rec"))):
                        tot_all = lp.tile([128, NT], dt.float32, name="tot_all", bufs=1)
                        pos_all = lp.tile([128, NT], dt.float32, name="pos_all", bufs=1)
                        for t in range(NT):
                            tot_cols = lp.tile([128, NJ], dt.float32, name="tot_cols")
                            pos_cols = lp.tile([128, NJ], dt.float32, name="pos_cols")
                            lhs0 = znt_own[e][:, 0, t * 128:(t + 1) * 128]
                            lhs1 = znt_own[e][:, 1, t * 128:(t + 1) * 128]
                            for jb in range(NJ):
                                sim_ps = psl.tile([128, JW], dt.float32, name="sim_ps",
                                                  tag="sim", bufs=3)
                                j0 = jb * JW
                                nc.tensor.matmul(sim_ps[:, 0:512], lhs0,
                                                 znt_sb[e][:, 0, j0:j0 + 512],
                                                 start=True, stop=False)
                                nc.tensor.matmul(sim_ps[:, 512:1024], lhs0,
                                                 znt_sb[e][:, 0, j0 + 512:j0 + 1024],
                                                 start=True, stop=False)
                                nc.tensor.matmul(sim_ps[:, 0:512], lhs1,
                                                 znt_sb[e][:, 1, j0:j0 + 512],
                                                 start=False, stop=True)
                                nc.tensor.matmul(sim_ps[:, 512:1024], lhs1,
                                                 znt_sb[e][:, 1, j0 + 512:j0 + 1024],
                                                 start=False, stop=True)
                                refl = lp.tile([128, JW], dt.bfloat16, name="refl")
                                nc.scalar.activation(refl[:], sim_ps[:], AF.Exp,
                                                     accum_out=tot_cols[:, jb:jb + 1])
                                adj_t = lp.tile([128, JW], dt.bfloat16, name="adj_t")
                                nc.sync.dma_start(
                                    adj_t[:],
                                    adj_in[akey][t * 128:(t + 1) * 128, j0:j0 + JW])
                                mscr = lp.tile([128, JW], dt.bfloat16, name="mscr")
                                nc.vector._custom_dve(
                                    TENSOR_TENSOR_REDUCE, out=mscr[:], in0=refl[:],
                                    in1=adj_t[:], s0=0.0, s1=1.0,
                                    accum_out=pos_cols[:, jb:jb + 1])
                            nc.vector.reduce_sum(tot_all[:, t:t + 1], tot_cols[:],
                                                 axis=mybir.AxisListType.X)
                            nc.vector.reduce_sum(pos_all[:, t:t + 1], pos_cols[:],
                                                 axis=mybir.AxisListType.X)
                        # loss partial: sum_n ln(neg+sig) - ln(pos+sig)
                        neg = lp.tile([128, NT], dt.float32, name="neg", bufs=1)
                        nc.vector.tensor_tensor(out=neg[:], in0=tot_all[:], in1=pos_all[:],
                                                op=AL.subtract)
                        if debug:
                            psdbg = work.tile([128, NT], dt.float32, name="psdbg")
                            nc.vector.tensor_copy(psdbg[:], pos_all[:])
                            nc.sync.dma_start(dbg["pt"][il, 0], psdbg[:])
                            ttd = work.tile([128, NT], dt.float32, name="ttd")
                            nc.vector.tensor_copy(ttd[:], tot_all[:])
                            nc.sync.dma_start(dbg["pt"][il, 1], ttd[:])
                        nc.vector.tensor_scalar(out=pos_all[:], in0=pos_all[:],
                                                scalar1=SIGMA, scalar2=None, op0=AL.add)
                        nc.vector.tensor_scalar(out=neg[:], in0=neg[:],
                                                scalar1=SIGMA, scalar2=None, op0=AL.add)
                        nc.scalar.activation(pos_all[:], pos_all[:], AF.Ln)
                        nc.scalar.activation(neg[:], neg[:], AF.Ln)
                        dl = lp.tile([128, NT], dt.float32, name="dl", bufs=1)
                        nc.vector.tensor_tensor(out=dl[:], in0=neg[:], in1=pos_all[:],
                                                op=AL.subtract)
                        nc.vector.reduce_sum(loss_parts[:, il:il + 1], dl[:],
                                             axis=mybir.AxisListType.X)

                # =======================================================
                # dim_lable_loss part 2: dim_center + refl2
                # =======================================================
                psd2cm = tc.tile_pool(name="psd2", bufs=1, space="PSUM")
                psd = psd2cm.__enter__()
                dimf = dp.tile([128, 4, OUT + 1], dt.float32, bufs=1)
                nc.sync.dma_start(dimf[:], dim_full.rearrange("m p f -> p m f"))

                dcnT = dp.tile([128, 2, 512], dt.bfloat16, bufs=1)
                for mt in range(4):
                    csum = dp.tile([128, 1], dt.float32, name="csum")
                    nc.vector.tensor_scalar(out=csum[:], in0=dimf[:, mt, OUT:OUT + 1],
                                            scalar1=1e-5, scalar2=None, op0=AL.add)
                    nc.vector.reciprocal(csum[:], csum[:])
                    dc_t = dp.tile([128, OUT], dt.bfloat16, name="dc_t")
                    nc.vector.tensor_scalar(out=dc_t[:], in0=dimf[:, mt, 0:OUT],
                                            scalar1=csum[:], scalar2=None, op0=AL.mult)
                    if debug:
                        dcd = work.tile([128, OUT], dt.float32, name="dcd")
                        nc.vector.tensor_copy(dcd[:], dc_t[:])
                        nc.sync.dma_start(dbg["dc"][mt], dcd[:])
                    nrm2 = dp.tile([128, 1], dt.float32, name="nrm2d")
                    scr = dp.tile([128, OUT], dt.bfloat16, name="scrd")
                    nc.vector._custom_dve(TENSOR_TENSOR_REDUCE, out=scr[:],
                                          in0=dc_t[:], in1=dc_t[:], s0=0.0, s1=1.0,
                                          accum_out=nrm2[:])
                    nc.vector.tensor_scalar(out=nrm2[:], in0=nrm2[:], scalar1=1e-30,
                                            scalar2=None, op0=AL.max)
                    nc.scalar.activation(nrm2[:], nrm2[:], AF.Ln)
                    nc.scalar.activation(nrm2[:], nrm2[:], AF.Exp, scale=0.5)
                    nc.vector.tensor_scalar(out=nrm2[:], in0=nrm2[:], scalar1=1e-12,
                                            scalar2=None, op0=AL.max)
                    nc.vector.reciprocal(nrm2[:], nrm2[:])
                    nc.vector.tensor_scalar(out=dc_t[:], in0=dc_t[:], scalar1=nrm2[:],
                                            scalar2=None, op0=AL.mult)
                    for kc in range(2):
                        dct_ps = psd.tile([128, 128], dt.bfloat16, name="dct_ps",
                                          tag="dct", bufs=2)
                        nc.tensor.transpose(dct_ps[:], dc_t[:, kc * 128:(kc + 1) * 128],
                                            idbf_sb[:])
                        nc.vector.tensor_copy(dcnT[:, kc, mt * 128:(mt + 1) * 128],
                                              dct_ps[:])

                # refl2 = exp(zfuse_n @ dcn^T); pos/neg with X_hot mask
                tot2 = dp.tile([128, NT], dt.float32, bufs=1)
                pos2 = dp.tile([128, NT], dt.float32, bufs=1)
                for t in range(NT):
                    r2_ps = psd.tile([128, 512], dt.float32, name="r2_ps",
                                     tag="xtz", bufs=2)
                    nc.tensor.matmul(r2_ps[:], znt_own["zf"][:, 0, t * 128:(t + 1) * 128],
                                     dcnT[:, 0, :], start=True, stop=False)
                    nc.tensor.matmul(r2_ps[:], znt_own["zf"][:, 1, t * 128:(t + 1) * 128],
                                     dcnT[:, 1, :], start=False, stop=True)
                    refl2 = dp.tile([128, 512], dt.bfloat16, name="refl2")
                    nc.scalar.activation(refl2[:], r2_ps[:], AF.Exp,
                                         accum_out=tot2[:, t:t + 1])
                    xhot = dp.tile([128, 512], dt.bfloat16, name="xhot")
                    nc.vector.tensor_scalar(out=xhot[:], in0=xblk_sb[:, t, :],
                                            scalar1=0.0, scalar2=None, op0=AL.is_gt)
                    scr2 = dp.tile([128, 512], dt.bfloat16, name="scr2")
                    nc.vector._custom_dve(TENSOR_TENSOR_REDUCE, out=scr2[:],
                                          in0=refl2[:], in1=xhot[:], s0=0.0, s1=1.0,
                                          accum_out=pos2[:, t:t + 1])
                if debug:
                    p2d = work.tile([128, NT], dt.float32, name="p2d")
                    nc.vector.tensor_copy(p2d[:], pos2[:])
                    nc.sync.dma_start(dbg["pt2"][0], p2d[:])
                    t2d = work.tile([128, NT], dt.float32, name="t2d")
                    nc.vector.tensor_copy(t2d[:], tot2[:])
                    nc.sync.dma_start(dbg["pt2"][1], t2d[:])
                # loss_feat partial: -ln(pos/neg + 1e-5), pos=pos2+SIG, neg=tot2-pos2
                neg2 = dp.tile([128, NT], dt.float32, bufs=1)
                nc.vector.tensor_tensor(out=neg2[:], in0=tot2[:], in1=pos2[:],
                                        op=AL.subtract)
                nc.vector.tensor_scalar(out=pos2[:], in0=pos2[:], scalar1=SIGMA,
                                        scalar2=None, op0=AL.add)
                nc.vector.reciprocal(neg2[:], neg2[:])
                r = dp.tile([128, NT], dt.float32, bufs=1)
                nc.vector.tensor_tensor(out=r[:], in0=pos2[:], in1=neg2[:], op=AL.mult)
                nc.vector.tensor_scalar(out=r[:], in0=r[:], scalar1=1e-5,
                                        scalar2=None, op0=AL.add)
                nc.scalar.activation(r[:], r[:], AF.Ln)
                rsum = dp.tile([128, 1], dt.float32, bufs=1)
                nc.vector.reduce_sum(rsum[:], r[:], axis=mybir.AxisListType.X)
                nc.vector.tensor_scalar(out=loss_parts[:, 3:4], in0=rsum[:],
                                        scalar1=-1.0, scalar2=None, op0=AL.mult)
                psd2cm.__exit__(None, None, None)

            # ---------- output + end barrier ----------
            nc.sync.dma_start(out_t[:], loss_parts[:])
            barc = statp.tile([128, 1], dt.float32)
            nc.vector.reduce_sum(barc[:], loss_parts[:], axis=mybir.AxisListType.X)
            nc.sync.dma_start(bar_in[:], barc[:])
            nc.gpsimd.collective_compute(
                "AllReduce", AL.add, replica_groups=RG,
                ins=[bar_in[:]], outs=[bar_out[:]])

    nc.compile()
    return nc


# ---------------------------------------------------------------- entry point
def _prep(feat, adj_label, adj_X, adj_rec, W0a, b0a, W1a, b1a,
          W0x, b0x, W1x, b1x, Wp1, bp1, wp2, edge_index, edge_index_x,
          _debug=False):
    feat = np.asarray(feat, np.float32)
    ga = _prep_graph(np.asarray(edge_index))
    gx = _prep_graph(np.asarray(edge_index_x))

    key = (ga["nb"], gx["nb"], _debug)
    if key not in _cache:
        _cache[key] = _build(*key[:2], debug=_debug)
    nc = _cache[key]

    feat_bf = feat.astype(BF16)
    iota = np.tile(np.arange(128, dtype=np.float32)[None, :], (128, 1)).astype(BF16)
    idbf = np.eye(128, dtype=np.float32).astype(BF16)

    base = dict(
        feat_bf=feat_bf, iota=iota, idbf=idbf,
        W0a=np.asarray(W0a, np.float32).astype(BF16),
        W1a=np.asarray(W1a, np.float32).astype(BF16),
        b0a=np.asarray(b0a, np.float32).reshape(1, HID).astype(BF16),
        b1a=np.asarray(b1a, np.float32).reshape(1, OUT).astype(BF16),
        W0x=np.asarray(W0x, np.float32).astype(BF16),
        W1x=np.asarray(W1x, np.float32).astype(BF16),
        b0x=np.asarray(b0x, np.float32).reshape(1, HID).astype(BF16),
        b1x=np.asarray(b1x, np.float32).reshape(1, OUT).astype(BF16),
        Wp1=np.asarray(Wp1, np.float32).astype(BF16),
        bp1=np.asarray(bp1, np.float32).reshape(1, ATT_H).astype(BF16),
        wp2=np.asarray(wp2, np.float32).astype(BF16),
    )
    adj_bf = {k: np.asarray(v, np.float32).astype(BF16)
              for k, v in (("label", adj_label), ("X", adj_X), ("rec", adj_rec))}

    in_maps = []
    for c in range(NC_):
        m = dict(base)
        m["xblk"] = feat_bf[c * ROWS:(c + 1) * ROWS]
        for k in ("label", "X", "rec"):
            m[f"adj_{k}"] = np.ascontiguousarray(adj_bf[k][c * ROWS:(c + 1) * ROWS])
        for gname, g in (("a", ga), ("x", gx)):
            m[f"srcidx_{gname}"] = g["src_idx"][c]
            m[f"dstid_{gname}"] = g["dst_ids"][c]
            m[f"sval_{gname}"] = g["sval"][c]
            m[f"nd_{gname}"] = g["nd"][c]
        in_maps.append(m)

    return nc, in_maps


def kernel(_debug=False, _trace=False, _tmpdir=None, **inputs):
    from concourse.bass_utils import run_bass_kernel_spmd
    nc, in_maps = _prep(_debug=_debug, **inputs)
    res = run_bass_kernel_spmd(nc, in_maps, core_ids=list(range(NC_)), trace=_trace,
                               tmpdir=_tmpdir)
    parts = np.stack([r["out"] for r in res.results])  # [8, 128, 8]
    psum = parts.sum(axis=(0, 1))  # [8]
    la, lx, ladj, lf = psum[0] / N, psum[1] / N, psum[2] / N, psum[3] / N
    val = np.float32(LAM * (la + lx) + ALPHA * lf + ladj)
    if _debug or _trace:
        kernel._last = res
    return np.asarray(val, np.float32).reshape(())
